# revision 4
# baseline (speedup 1.0000x reference)
"""BipartiteSAGE (2-layer GraphSAGE on a bipartite graph) for 8 trn2 NeuronCores.

Strategy (dst-sharded, feature-major GEMMs):
- src rows sharded contiguously 1250/core; dst nodes assigned to 80 balanced
  (core, block) bins of 125 nodes via greedy binning so every 128-slot block
  has ~equal edge count.
- Layer-1 aggregation uses linearity: mean1_raw = segmean(x_src) gathered
  straight from a replicated bf16 copy of x_src; mean1 = mean1_raw @ W_src^T
  (+ b_src x [cnt>0] via a K=1 outer-product matmul). No collective needed.
- Layer-2 gathers from an AllGather of the post-BN/ReLU src rows.
- Aggregation on device: dma_gather (128 edges/partition-tile) followed by
  one-hot S-matmul accumulation into PSUM per 128-dst block.
- BatchNorm: local sum/sumsq reductions, AllReduce of [512,2] stats.
- All GEMMs bf16 (stationary = transposed weights), accumulation fp32 in PSUM.
"""

import numpy as np
import ml_dtypes

N_SRC, N_DST = 10000, 10000
IN_SRC, IN_DST, HID, OUT = 512, 256, 512, 256
N_EDGES = 160000
EPS = 1e-5
NC_ = 8            # cores
NB = 10            # dst blocks per core
CAP = 125          # dst nodes per bin
LOC = 1280         # padded local columns per half (src / dst)
COLS = 2 * LOC
SRC_LOC = N_SRC // NC_   # 1250


def _preprocess(edge_index):
    """Balanced dst binning + per-core edge tiles. Returns static structures."""
    src = np.asarray(edge_index[0], dtype=np.int64)
    dst = np.asarray(edge_index[1], dtype=np.int64) - N_SRC
    cnt = np.bincount(dst, minlength=N_DST)

    import heapq
    nbins = NC_ * NB
    order = np.argsort(-cnt, kind="stable")
    heap = [(0, b) for b in range(nbins)]
    heapq.heapify(heap)
    bin_nodes = [[] for _ in range(nbins)]
    bin_load = [0] * nbins
    for node in order:
        while True:
            load, b = heapq.heappop(heap)
            if len(bin_nodes[b]) < CAP:
                break
        bin_nodes[b].append(int(node))
        bin_load[b] = load + int(cnt[node])
        heapq.heappush(heap, (bin_load[b], b))

    # node -> (bin, slot)
    bin_of = np.empty(N_DST, np.int64)
    slot_of = np.empty(N_DST, np.int64)
    for b, nodes in enumerate(bin_nodes):
        for s, nd in enumerate(nodes):
            bin_of[nd] = b
            slot_of[nd] = s

    # group edges by bin, sort by src within bin
    ebin = bin_of[dst]
    order_e = np.lexsort((src, ebin))
    src_s, dst_s, ebin_s = src[order_e], dst[order_e], ebin[order_e]
    bounds = np.searchsorted(ebin_s, np.arange(nbins + 1))
    max_edges = max(bounds[b + 1] - bounds[b] for b in range(nbins))
    TB = int(np.ceil(max_edges / 128))           # tiles per block
    TB += TB % 2                                 # even (half-block gathers)
    T = NB * TB                                  # tiles per core

    idx1 = np.zeros((NC_, T * 128), np.int16)
    idx2 = np.zeros((NC_, T * 128), np.int16)
    S = np.zeros((NC_, 128, T, 128), ml_dtypes.bfloat16)
    recip = np.ones((NC_, 128, NB, 1), np.float32)
    mask = np.zeros((NC_, 1, LOC), ml_dtypes.bfloat16)

    for b in range(nbins):
        c, blk = divmod(b, NB)
        e0, e1 = bounds[b], bounds[b + 1]
        ss, dd = src_s[e0:e1], dst_s[e0:e1]
        n = e1 - e0
        base = blk * TB * 128
        idx1[c, base:base + n] = ss.astype(np.int16)
        idx2[c, base:base + n] = (1280 * (ss // SRC_LOC) + ss % SRC_LOC).astype(np.int16)
        pos = np.arange(n)
        S[c, pos % 128, blk * TB + pos // 128, slot_of[dd]] = 1.0
        for s, nd in enumerate(bin_nodes[b]):
            recip[c, s, blk, 0] = 1.0 / max(int(cnt[nd]), 1)
            if cnt[nd] > 0:
                mask[c, 0, blk * 128 + s] = 1.0

    def wrap(ix):  # flat [n] -> [128, n//16] wrapped-16 + replicated
        n = ix.shape[0]
        w = ix.reshape(n // 16, 16).T          # [16, n//16]
        return np.tile(w, (8, 1)).copy()

    idx1_w = np.stack([wrap(idx1[c]) for c in range(NC_)])
    idx2_w = np.stack([wrap(idx2[c]) for c in range(NC_)])
    return dict(TB=TB, T=T, bin_nodes=bin_nodes, cnt=cnt,
                idx1=idx1_w, idx2=idx2_w, S=S, recip=recip, mask=mask)


def _feat_major(v, kt):
    """[F] -> [128, kt, 1] f32 feature-major (f = t*128+p)."""
    return np.ascontiguousarray(
        np.asarray(v, np.float32).reshape(kt, 128, 1).transpose(1, 0, 2))


def _w_tiles(w):
    """W [out, in] -> lhsT tiles [128, in//128, out] bf16 (k = t*128+p)."""
    wt = np.asarray(w, np.float32).T           # [in, out]
    kin, kout = wt.shape
    return np.ascontiguousarray(
        wt.reshape(kin // 128, 128, kout).transpose(1, 0, 2)).astype(ml_dtypes.bfloat16)


def _x_tiles(x, ncols):
    """x [rows, F] -> rhs tiles [128, F//128, ncols] bf16 (feature-major, padded)."""
    r, f = x.shape
    xt = np.zeros((f, ncols), np.float32)
    xt[:, :r] = np.asarray(x, np.float32).T
    return np.ascontiguousarray(
        xt.reshape(f // 128, 128, ncols).transpose(1, 0, 2)).astype(ml_dtypes.bfloat16)


_BUILD_CACHE = {}


def _build(TB):
    import concourse.bacc as bacc
    import concourse.mybir as mybir
    from concourse import tile

    dt = mybir.dt
    T = NB * TB
    GH = TB * 64                  # idxs per half-block gather
    CH = [(0, 512), (512, 512), (1024, 256)]   # chunks over a 1280 half

    nc = bacc.Bacc("TRN2", target_bir_lowering=False, debug=False, num_devices=NC_)

    # ---- external inputs ----
    x_src_bf = nc.dram_tensor("x_src_bf", [N_SRC, 512], dt.bfloat16, kind="ExternalInput")
    xsT_d = nc.dram_tensor("xsT", [128, 4, LOC], dt.bfloat16, kind="ExternalInput")
    xdT_d = nc.dram_tensor("xdT", [128, 2, LOC], dt.bfloat16, kind="ExternalInput")
    wsrcT_d = nc.dram_tensor("wsrcT", [128, 4, 512], dt.bfloat16, kind="ExternalInput")
    wdstT_d = nc.dram_tensor("wdstT", [128, 2, 512], dt.bfloat16, kind="ExternalInput")
    w1lT_d = nc.dram_tensor("w1lT", [128, 4, 512], dt.bfloat16, kind="ExternalInput")
    w1rT_d = nc.dram_tensor("w1rT", [128, 4, 512], dt.bfloat16, kind="ExternalInput")
    w2lT_d = nc.dram_tensor("w2lT", [128, 4, 256], dt.bfloat16, kind="ExternalInput")
    w2rT_d = nc.dram_tensor("w2rT", [128, 4, 256], dt.bfloat16, kind="ExternalInput")
    S_d = nc.dram_tensor("S", [128, T, 128], dt.bfloat16, kind="ExternalInput")
    idx1_d = nc.dram_tensor("idx1", [128, T * 8], dt.int16, kind="ExternalInput")
    idx2_d = nc.dram_tensor("idx2", [128, T * 8], dt.int16, kind="ExternalInput")
    recip_d = nc.dram_tensor("recip", [128, NB, 1], dt.float32, kind="ExternalInput")
    mask_d = nc.dram_tensor("mask", [1, LOC], dt.bfloat16, kind="ExternalInput")
    bsrcl_d = nc.dram_tensor("bsrcl", [1, 512], dt.bfloat16, kind="ExternalInput")
    bsrc_d = nc.dram_tensor("bsrc", [128, 4, 1], dt.float32, kind="ExternalInput")
    bdst_d = nc.dram_tensor("bdst", [128, 4, 1], dt.float32, kind="ExternalInput")
    gamma_d = nc.dram_tensor("gamma", [128, 4, 1], dt.float32, kind="ExternalInput")
    beta_d = nc.dram_tensor("beta", [128, 4, 1], dt.float32, kind="ExternalInput")
    b2_d = nc.dram_tensor("b2", [128, 2, 1], dt.float32, kind="ExternalInput")
    out_d = nc.dram_tensor("outT", [128, 2, COLS], dt.float32, kind="ExternalOutput")

    RG = [list(range(NC_))]
    AF = mybir.ActivationFunctionType
    ALU = mybir.AluOpType

    with tile.TileContext(nc) as tc:
        with (
            tc.tile_pool(name="w", bufs=1) as wp,
            tc.tile_pool(name="st", bufs=1) as sp,
            tc.tile_pool(name="msgs", bufs=2) as mp,
            tc.tile_pool(name="mean", bufs=3) as meanp,
            tc.tile_pool(name="ps", bufs=5, space="PSUM") as pp,
            tc.tile_pool(name="pagg", bufs=2, space="PSUM") as pap,
            tc.tile_pool(name="dram", bufs=1, space="DRAM") as dp,
        ):
            def load(d, shape, dtype, pool=wp, tag=None):
                t_ = pool.tile(shape, dtype, tag=tag or f"ld_{d.name}")
                nc.sync.dma_start(t_[:], d[:])
                return t_

            # persistent loads
            idx1_t = load(idx1_d, [128, T * 8], dt.int16)
            idx2_t = load(idx2_d, [128, T * 8], dt.int16)
            S_t = load(S_d, [128, T, 128], dt.bfloat16)
            wsrcT = load(wsrcT_d, [128, 4, 512], dt.bfloat16)
            wdstT = load(wdstT_d, [128, 2, 512], dt.bfloat16)
            w1lT = load(w1lT_d, [128, 4, 512], dt.bfloat16)
            w1rT = load(w1rT_d, [128, 4, 512], dt.bfloat16)
            w2lT = load(w2lT_d, [128, 4, 256], dt.bfloat16)
            w2rT = load(w2rT_d, [128, 4, 256], dt.bfloat16)
            xsT = load(xsT_d, [128, 4, LOC], dt.bfloat16, tag="xsT_rows")
            xdT = load(xdT_d, [128, 2, LOC], dt.bfloat16)
            recip_t = load(recip_d, [128, NB, 1], dt.float32)
            mask_t = load(mask_d, [1, LOC], dt.bfloat16)
            bsrcl_t = load(bsrcl_d, [1, 512], dt.bfloat16)
            bsrc_t = load(bsrc_d, [128, 4, 1], dt.float32)
            bdst_t = load(bdst_d, [128, 4, 1], dt.float32)
            gamma_t = load(gamma_d, [128, 4, 1], dt.float32)
            beta_t = load(beta_d, [128, 4, 1], dt.float32)
            b2_t = load(b2_d, [128, 2, 1], dt.float32)

            # big stream buffers
            hT = sp.tile([128, 4, COLS], dt.bfloat16, tag="actT")      # h feature-major
            r1T = sp.tile([128, 4, LOC], dt.float32, tag="bigf32a")    # x1 src half (pre-BN)
            m1rT = sp.tile([128, 4, LOC], dt.bfloat16, tag="mT")       # raw mean1^T
            m1T = sp.tile([128, 4, LOC], dt.bfloat16, tag="m1T")       # mean1 @ WsrcT
            x1dT = sp.tile([128, 4, LOC], dt.float32, tag="x1dT")      # x1 dst half (pre-BN)

            # ---------- h^T = [W_src x_src^T | W_dst x_dst^T] + biases ----------
            for t in range(4):
                for cs, cw in CH:
                    ps = pp.tile([128, 512], dt.float32, tag="pgemm")
                    for k in range(4):
                        nc.tensor.matmul(ps[:, :cw], wsrcT[:, k, t * 128:(t + 1) * 128],
                                         xsT[:, k, cs:cs + cw], start=(k == 0), stop=(k == 3))
                    nc.scalar.activation(hT[:, t, cs:cs + cw], ps[:, :cw], AF.Identity,
                                         bias=bsrc_t[:, t, :], scale=1.0)
            for t in range(4):
                for cs, cw in CH:
                    ps = pp.tile([128, 512], dt.float32, tag="pgemm")
                    for k in range(2):
                        nc.tensor.matmul(ps[:, :cw], wdstT[:, k, t * 128:(t + 1) * 128],
                                         xdT[:, k, cs:cs + cw], start=(k == 0), stop=(k == 1))
                    nc.scalar.activation(hT[:, t, LOC + cs:LOC + cs + cw], ps[:, :cw],
                                         AF.Identity, bias=bdst_t[:, t, :], scale=1.0)

            # ---------- r1^T src half = W1r h^T (src cols), f32 ----------
            for t in range(4):
                for cs, cw in CH:
                    ps = pp.tile([128, 512], dt.float32, tag="pgemm")
                    for k in range(4):
                        nc.tensor.matmul(ps[:, :cw], w1rT[:, k, t * 128:(t + 1) * 128],
                                         hT[:, k, cs:cs + cw], start=(k == 0), stop=(k == 3))
                    nc.vector.tensor_copy(r1T[:, t, cs:cs + cw], ps[:, :cw])

            # ---------- layer-1 aggregation: gather + S-matmul per block ----------
            def aggregate(idx_t, src_dram, outT, layer):
                # outT [128, 4, LOC] bf16 feature-major result (transposed mean)
                for b in range(NB):
                    pa = pap.tile([128, 512], dt.float32, tag="pagg")
                    for h in range(2):
                        ms = mp.tile([128, TB // 2, 512], dt.bfloat16, tag="msgs")
                        nc.gpsimd.dma_gather(
                            ms[:], src_dram[:],
                            idx_t[:, b * TB * 8 + h * TB * 4:b * TB * 8 + (h + 1) * TB * 4],
                            GH, GH, 512)
                        for j in range(TB // 2):
                            jj = h * (TB // 2) + j
                            nc.tensor.matmul(pa[:], S_t[:, b * TB + jj, :], ms[:, j, :],
                                             start=(jj == 0), stop=(jj == TB - 1))
                    mb = meanp.tile([128, 512], dt.bfloat16, tag="meanblk")
                    nc.vector.tensor_scalar_mul(mb[:], pa[:], recip_t[:, b, :])
                    for t in range(4):
                        nc.sync.dma_start_transpose(
                            outT[:, t, b * 128:(b + 1) * 128], mb[:, t * 128:(t + 1) * 128])

            aggregate(idx1_t, x_src_bf, m1rT, 1)

            # ---------- mean1^T = W_src m1r^T + b_src (x) mask ----------
            for t in range(4):
                for cs, cw in CH:
                    ps = pp.tile([128, 512], dt.float32, tag="pgemm")
                    for k in range(4):
                        nc.tensor.matmul(ps[:, :cw], wsrcT[:, k, t * 128:(t + 1) * 128],
                                         m1rT[:, k, cs:cs + cw], start=(k == 0), stop=False)
                    nc.tensor.matmul(ps[:, :cw], bsrcl_t[0:1, t * 128:(t + 1) * 128],
                                     mask_t[0:1, cs:cs + cw], start=False, stop=True)
                    nc.vector.tensor_copy(m1T[:, t, cs:cs + cw], ps[:, :cw])

            # ---------- x1 dst half = W1r h^T(dst) + W1l mean1^T ----------
            for t in range(4):
                for cs, cw in CH:
                    ps = pp.tile([128, 512], dt.float32, tag="pgemm")
                    for k in range(4):
                        nc.tensor.matmul(ps[:, :cw], w1rT[:, k, t * 128:(t + 1) * 128],
                                         hT[:, k, LOC + cs:LOC + cs + cw], start=(k == 0), stop=False)
                    for k in range(4):
                        nc.tensor.matmul(ps[:, :cw], w1lT[:, k, t * 128:(t + 1) * 128],
                                         m1T[:, k, cs:cs + cw], start=False, stop=(k == 3))
                    nc.vector.tensor_copy(x1dT[:, t, cs:cs + cw], ps[:, :cw])

            # ---------- BN stats: S1, S2 over real columns ----------
            stats = sp.tile([128, 4, 4], dt.float32, tag="stats")   # s1s, s1d, s2s, s2d
            arin_sb = sp.tile([128, 4, 2], dt.float32, tag="arin")
            sq = sp.tile([128, LOC], dt.bfloat16, tag="sqscratch")
            for t in range(4):
                dst_real = x1dT[:, t, :].rearrange("p (b s) -> p b s", b=NB)[:, :, 0:CAP]
                sq_dst = sq[:, :].rearrange("p (b s) -> p b s", b=NB)[:, :, 0:CAP]
                nc.vector.tensor_reduce(stats[:, t, 0:1], r1T[:, t, 0:SRC_LOC],
                                        mybir.AxisListType.X, ALU.add)
                nc.vector.tensor_reduce(stats[:, t, 1:2], dst_real,
                                        mybir.AxisListType.XY, ALU.add)
                nc.scalar.activation(sq[:, 0:SRC_LOC], r1T[:, t, 0:SRC_LOC], AF.Square,
                                     accum_out=stats[:, t, 2:3])
                nc.scalar.activation(sq_dst, dst_real, AF.Square,
                                     accum_out=stats[:, t, 3:4])
                nc.vector.tensor_tensor(arin_sb[:, t, 0:1], stats[:, t, 0:1],
                                        stats[:, t, 1:2], ALU.add)
                nc.vector.tensor_tensor(arin_sb[:, t, 1:2], stats[:, t, 2:3],
                                        stats[:, t, 3:4], ALU.add)

            ar_in = dp.tile([128, 8], dt.float32)
            ar_out = dp.tile([128, 8], dt.float32, addr_space="Shared")
            nc.sync.dma_start(ar_in[:], arin_sb[:].rearrange("p a b -> p (a b)"))
            nc.gpsimd.collective_compute("AllReduce", ALU.add, replica_groups=RG,
                                         ins=[ar_in[:]], outs=[ar_out[:]])
            arsum = sp.tile([128, 4, 2], dt.float32, tag="arsum")
            nc.sync.dma_start(arsum[:], ar_out[:].rearrange("p (a b) -> p a b", a=4))

            # mean/var -> scale a, bias bb  (all [128, 4, 1] feature-major)
            mean_v = sp.tile([128, 4, 1], dt.float32, tag="vec1")
            var_v = sp.tile([128, 4, 1], dt.float32, tag="vec2")
            av = sp.tile([128, 4, 1], dt.float32, tag="vec3")
            bv = sp.tile([128, 4, 1], dt.float32, tag="vec4")
            inv_n = 1.0 / (N_SRC + N_DST)
            nc.vector.tensor_scalar_mul(mean_v[:], arsum[:, :, 0:1], inv_n)
            nc.vector.tensor_scalar_mul(var_v[:], arsum[:, :, 1:2], inv_n)   # E[x^2]
            nc.vector.tensor_tensor(av[:], mean_v[:], mean_v[:], ALU.mult)   # mean^2
            nc.vector.tensor_tensor(var_v[:], var_v[:], av[:], ALU.subtract)
            nc.vector.tensor_scalar_add(var_v[:], var_v[:], EPS)
            for t in range(4):
                nc.scalar.activation(var_v[:, t, :], var_v[:, t, :], AF.Sqrt, bias=0.0)
            nc.vector.reciprocal(var_v[:], var_v[:])                          # 1/std
            nc.vector.tensor_tensor(av[:], gamma_t[:], var_v[:], ALU.mult)    # a
            nc.vector.tensor_tensor(bv[:], mean_v[:], av[:], ALU.mult)
            nc.vector.tensor_tensor(bv[:], beta_t[:], bv[:], ALU.subtract)    # beta - mean*a

            # ---------- x1' = relu(a*x1 + b), bf16 (reuse hT slot via tag) ----------
            x1pT = sp.tile([128, 4, COLS], dt.bfloat16, tag="actT")
            for t in range(4):
                nc.scalar.activation(x1pT[:, t, 0:LOC], r1T[:, t, :], AF.Relu,
                                     bias=bv[:, t, :], scale=av[:, t, :])
                nc.scalar.activation(x1pT[:, t, LOC:COLS], x1dT[:, t, :], AF.Relu,
                                     bias=bv[:, t, :], scale=av[:, t, :])

            # ---------- transpose src half, AllGather ----------
            x1rows = sp.tile([128, NB, 512], dt.bfloat16, tag="xsT_rows")
            for t in range(4):
                for r in range(NB):
                    nc.sync.dma_start_transpose(
                        x1rows[:, r, t * 128:(t + 1) * 128], x1pT[:, t, r * 128:(r + 1) * 128])
            ag_in = dp.tile([LOC, 512], dt.bfloat16)
            ag_out = dp.tile([NC_ * LOC, 512], dt.bfloat16, addr_space="Shared")
            nc.sync.dma_start(ag_in[:].rearrange("(r p) f -> p r f", p=128), x1rows[:])
            nc.gpsimd.collective_compute("AllGather", ALU.bypass, replica_groups=RG,
                                         ins=[ag_in[:]], outs=[ag_out[:]])

            # ---------- layer-2 aggregation ----------
            m2T = sp.tile([128, 4, LOC], dt.bfloat16, tag="mT")
            aggregate(idx2_t, ag_out, m2T, 2)

            # ---------- output GEMMs ----------
            outT = sp.tile([128, 2, COLS], dt.float32, tag="bigf32a")
            for o in range(2):
                for cs, cw in CH:
                    ps = pp.tile([128, 512], dt.float32, tag="pgemm")
                    for k in range(4):
                        nc.tensor.matmul(ps[:, :cw], w2rT[:, k, o * 128:(o + 1) * 128],
                                         x1pT[:, k, cs:cs + cw], start=(k == 0), stop=(k == 3))
                    nc.scalar.activation(outT[:, o, cs:cs + cw], ps[:, :cw], AF.Identity,
                                         bias=b2_t[:, o, :], scale=1.0)
            for o in range(2):
                for cs, cw in CH:
                    ps = pp.tile([128, 512], dt.float32, tag="pgemm")
                    for k in range(4):
                        nc.tensor.matmul(ps[:, :cw], w2rT[:, k, o * 128:(o + 1) * 128],
                                         x1pT[:, k, LOC + cs:LOC + cs + cw],
                                         start=(k == 0), stop=False)
                    for k in range(4):
                        nc.tensor.matmul(ps[:, :cw], w2lT[:, k, o * 128:(o + 1) * 128],
                                         m2T[:, k, cs:cs + cw], start=False, stop=(k == 3))
                    nc.scalar.activation(outT[:, o, LOC + cs:LOC + cs + cw], ps[:, :cw],
                                         AF.Identity, bias=b2_t[:, o, :], scale=1.0)
            nc.sync.dma_start(out_d[:], outT[:])

    nc.compile()
    return nc


def kernel(**inputs):
    from concourse.bass_utils import run_bass_kernel_spmd

    x_src = np.asarray(inputs["x_src"], np.float32)
    x_dst = np.asarray(inputs["x_dst"], np.float32)
    edge_index = np.asarray(inputs["edge_index"])
    pre = _preprocess(edge_index)
    TB = pre["TB"]

    key = TB
    if key not in _BUILD_CACHE:
        _BUILD_CACHE[key] = _build(TB)
    nc = _BUILD_CACHE[key]

    x_src_bf = np.ascontiguousarray(x_src).astype(ml_dtypes.bfloat16)
    wsrcT = _w_tiles(inputs["W_src"])
    wdstT = _w_tiles(inputs["W_dst"])
    w1lT = _w_tiles(inputs["W1l"])
    w1rT = _w_tiles(inputs["W1r"])
    w2lT = _w_tiles(inputs["W2l"])
    w2rT = _w_tiles(inputs["W2r"])
    bsrc = _feat_major(inputs["b_src"], 4)
    bdst = _feat_major(inputs["b_dst"], 4)
    gamma = _feat_major(inputs["gamma"], 4)
    beta = _feat_major(inputs["beta"], 4)
    b2 = _feat_major(inputs["b2"], 2)
    bsrcl = np.asarray(inputs["b_src"], np.float32).reshape(1, 512).astype(ml_dtypes.bfloat16)

    in_maps = []
    for c in range(NC_):
        xs = x_src[c * SRC_LOC:(c + 1) * SRC_LOC]
        nodes = [nd for b in range(NB) for nd in
                 (pre["bin_nodes"][c * NB + b] + [None] * (128 - len(pre["bin_nodes"][c * NB + b])))]
        xd = np.zeros((LOC, IN_DST), np.float32)
        for col, nd in enumerate(nodes):
            if nd is not None:
                xd[col] = x_dst[nd]
        in_maps.append({
            "x_src_bf": x_src_bf,
            "xsT": _x_tiles(xs, LOC),
            "xdT": np.ascontiguousarray(
                xd.T.reshape(2, 128, LOC).transpose(1, 0, 2)).astype(ml_dtypes.bfloat16),
            "wsrcT": wsrcT, "wdstT": wdstT, "w1lT": w1lT, "w1rT": w1rT,
            "w2lT": w2lT, "w2rT": w2rT,
            "S": np.ascontiguousarray(pre["S"][c]),
            "idx1": pre["idx1"][c], "idx2": pre["idx2"][c],
            "recip": pre["recip"][c], "mask": pre["mask"][c],
            "bsrcl": bsrcl, "bsrc": bsrc, "bdst": bdst,
            "gamma": gamma, "beta": beta, "b2": b2,
        })

    res = run_bass_kernel_spmd(nc, in_maps, core_ids=list(range(NC_)))

    out = np.zeros((N_SRC + N_DST, OUT), np.float32)
    for c in range(NC_):
        arr = res.results[c]["outT"].transpose(1, 0, 2).reshape(OUT, COLS)  # [feat, col]
        out[c * SRC_LOC:(c + 1) * SRC_LOC] = arr[:, 0:SRC_LOC].T
        for b in range(NB):
            nodes = pre["bin_nodes"][c * NB + b]
            cols = LOC + b * 128 + np.arange(len(nodes))
            out[N_SRC + np.asarray(nodes, np.int64)] = arr[:, cols].T
    return out


# revision 5
# speedup vs baseline: 1.0547x; 1.0547x over previous
"""BipartiteSAGE (2-layer GraphSAGE on a bipartite graph) for 8 trn2 NeuronCores.

Strategy (dst-sharded, feature-major GEMMs):
- src rows sharded contiguously 1250/core; dst nodes assigned to 80 balanced
  (core, block) bins of 125 nodes via greedy binning so every 128-slot block
  has ~equal edge count.
- Layer-1 aggregation uses linearity: mean1_raw = segmean(x_src) gathered
  straight from a replicated bf16 copy of x_src; mean1 = mean1_raw @ W_src^T
  (+ b_src x [cnt>0] via a K=1 outer-product matmul). No collective needed.
- Layer-2 gathers from an AllGather of the post-BN/ReLU src rows.
- Aggregation on device: dma_gather (128 edges/partition-tile) followed by
  one-hot S-matmul accumulation into PSUM per 128-dst block.
- BatchNorm: local sum/sumsq reductions, AllReduce of [512,2] stats.
- All GEMMs bf16 (stationary = transposed weights), accumulation fp32 in PSUM.
"""

import numpy as np
import ml_dtypes

N_SRC, N_DST = 10000, 10000
IN_SRC, IN_DST, HID, OUT = 512, 256, 512, 256
N_EDGES = 160000
EPS = 1e-5
NC_ = 8            # cores
NB = 10            # dst blocks per core
CAP = 125          # dst nodes per bin
LOC = 1280         # padded local columns per half (src / dst)
COLS = 2 * LOC
SRC_LOC = N_SRC // NC_   # 1250


def _preprocess(edge_index):
    """Balanced dst binning + per-core edge tiles. Returns static structures."""
    src = np.asarray(edge_index[0], dtype=np.int64)
    dst = np.asarray(edge_index[1], dtype=np.int64) - N_SRC
    cnt = np.bincount(dst, minlength=N_DST)

    import heapq
    nbins = NC_ * NB
    order = np.argsort(-cnt, kind="stable")
    heap = [(0, b) for b in range(nbins)]
    heapq.heapify(heap)
    bin_nodes = [[] for _ in range(nbins)]
    bin_load = [0] * nbins
    for node in order:
        while True:
            load, b = heapq.heappop(heap)
            if len(bin_nodes[b]) < CAP:
                break
        bin_nodes[b].append(int(node))
        bin_load[b] = load + int(cnt[node])
        heapq.heappush(heap, (bin_load[b], b))

    # node -> (bin, slot)
    bin_of = np.empty(N_DST, np.int64)
    slot_of = np.empty(N_DST, np.int64)
    for b, nodes in enumerate(bin_nodes):
        for s, nd in enumerate(nodes):
            bin_of[nd] = b
            slot_of[nd] = s

    # group edges by bin, sort by src within bin
    ebin = bin_of[dst]
    order_e = np.lexsort((src, ebin))
    src_s, dst_s, ebin_s = src[order_e], dst[order_e], ebin[order_e]
    bounds = np.searchsorted(ebin_s, np.arange(nbins + 1))
    max_edges = max(bounds[b + 1] - bounds[b] for b in range(nbins))
    TB = int(np.ceil(max_edges / 128))           # tiles per block
    TB += TB % 2                                 # even (half-block gathers)
    T = NB * TB                                  # tiles per core

    idx1 = np.zeros((NC_, T * 128), np.int16)
    idx2 = np.zeros((NC_, T * 128), np.int16)
    S = np.zeros((NC_, 128, T, 128), ml_dtypes.bfloat16)
    recip = np.ones((NC_, 128, NB, 1), np.float32)
    mask = np.zeros((NC_, 1, LOC), ml_dtypes.bfloat16)

    for b in range(nbins):
        c, blk = divmod(b, NB)
        e0, e1 = bounds[b], bounds[b + 1]
        ss, dd = src_s[e0:e1], dst_s[e0:e1]
        n = e1 - e0
        base = blk * TB * 128
        idx1[c, base:base + n] = ss.astype(np.int16)
        idx2[c, base:base + n] = (1280 * (ss // SRC_LOC) + ss % SRC_LOC).astype(np.int16)
        pos = np.arange(n)
        S[c, pos % 128, blk * TB + pos // 128, slot_of[dd]] = 1.0
        for s, nd in enumerate(bin_nodes[b]):
            recip[c, s, blk, 0] = 1.0 / max(int(cnt[nd]), 1)
            if cnt[nd] > 0:
                mask[c, 0, blk * 128 + s] = 1.0

    def wrap(ix):  # flat [n] -> [128, n//16] wrapped-16 + replicated
        n = ix.shape[0]
        w = ix.reshape(n // 16, 16).T          # [16, n//16]
        return np.tile(w, (8, 1)).copy()

    idx1_w = np.stack([wrap(idx1[c]) for c in range(NC_)])
    idx2_w = np.stack([wrap(idx2[c]) for c in range(NC_)])
    return dict(TB=TB, T=T, bin_nodes=bin_nodes, cnt=cnt,
                idx1=idx1_w, idx2=idx2_w, S=S, recip=recip, mask=mask)


def _feat_major(v, kt):
    """[F] -> [128, kt, 1] f32 feature-major (f = t*128+p)."""
    return np.ascontiguousarray(
        np.asarray(v, np.float32).reshape(kt, 128, 1).transpose(1, 0, 2))


def _w_tiles(w):
    """W [out, in] -> lhsT tiles [128, in//128, out] bf16 (k = t*128+p)."""
    wt = np.asarray(w, np.float32).T           # [in, out]
    kin, kout = wt.shape
    return np.ascontiguousarray(
        wt.reshape(kin // 128, 128, kout).transpose(1, 0, 2)).astype(ml_dtypes.bfloat16)


def _x_tiles(x, ncols):
    """x [rows, F] -> rhs tiles [128, F//128, ncols] bf16 (feature-major, padded)."""
    r, f = x.shape
    xt = np.zeros((f, ncols), np.float32)
    xt[:, :r] = np.asarray(x, np.float32).T
    return np.ascontiguousarray(
        xt.reshape(f // 128, 128, ncols).transpose(1, 0, 2)).astype(ml_dtypes.bfloat16)


_BUILD_CACHE = {}


def _build(TB):
    import concourse.bacc as bacc
    import concourse.mybir as mybir
    from concourse import tile

    dt = mybir.dt
    T = NB * TB
    GH = TB * 64                  # idxs per half-block gather
    CH = [(0, 512), (512, 512), (1024, 256)]   # chunks over a 1280 half

    nc = bacc.Bacc("TRN2", target_bir_lowering=False, debug=False, num_devices=NC_,
                   num_swdge_queues=4)

    # ---- external inputs ----
    x_src_bf = nc.dram_tensor("x_src_bf", [N_SRC, 512], dt.bfloat16, kind="ExternalInput")
    xsT_d = nc.dram_tensor("xsT", [128, 4, LOC], dt.bfloat16, kind="ExternalInput")
    xdT_d = nc.dram_tensor("xdT", [128, 2, LOC], dt.bfloat16, kind="ExternalInput")
    wsrcT_d = nc.dram_tensor("wsrcT", [128, 4, 512], dt.bfloat16, kind="ExternalInput")
    wdstT_d = nc.dram_tensor("wdstT", [128, 2, 512], dt.bfloat16, kind="ExternalInput")
    w1lT_d = nc.dram_tensor("w1lT", [128, 4, 512], dt.bfloat16, kind="ExternalInput")
    w1rT_d = nc.dram_tensor("w1rT", [128, 4, 512], dt.bfloat16, kind="ExternalInput")
    w2lT_d = nc.dram_tensor("w2lT", [128, 4, 256], dt.bfloat16, kind="ExternalInput")
    w2rT_d = nc.dram_tensor("w2rT", [128, 4, 256], dt.bfloat16, kind="ExternalInput")
    S_d = nc.dram_tensor("S", [128, T, 128], dt.bfloat16, kind="ExternalInput")
    idx1_d = nc.dram_tensor("idx1", [128, T * 8], dt.int16, kind="ExternalInput")
    idx2_d = nc.dram_tensor("idx2", [128, T * 8], dt.int16, kind="ExternalInput")
    recip_d = nc.dram_tensor("recip", [128, NB, 1], dt.float32, kind="ExternalInput")
    mask_d = nc.dram_tensor("mask", [1, LOC], dt.bfloat16, kind="ExternalInput")
    bsrcl_d = nc.dram_tensor("bsrcl", [1, 512], dt.bfloat16, kind="ExternalInput")
    bsrc_d = nc.dram_tensor("bsrc", [128, 4, 1], dt.float32, kind="ExternalInput")
    bdst_d = nc.dram_tensor("bdst", [128, 4, 1], dt.float32, kind="ExternalInput")
    gamma_d = nc.dram_tensor("gamma", [128, 4, 1], dt.float32, kind="ExternalInput")
    beta_d = nc.dram_tensor("beta", [128, 4, 1], dt.float32, kind="ExternalInput")
    b2_d = nc.dram_tensor("b2", [128, 2, 1], dt.float32, kind="ExternalInput")
    out_d = nc.dram_tensor("outT", [128, 2, COLS], dt.float32, kind="ExternalOutput")

    RG = [list(range(NC_))]
    AF = mybir.ActivationFunctionType
    ALU = mybir.AluOpType

    with tile.TileContext(nc) as tc:
        with (
            tc.tile_pool(name="w", bufs=1) as wp,
            tc.tile_pool(name="st", bufs=1) as sp,
            tc.tile_pool(name="msgs", bufs=2) as mp,
            tc.tile_pool(name="mean", bufs=3) as meanp,
            tc.tile_pool(name="ps", bufs=5, space="PSUM") as pp,
            tc.tile_pool(name="pagg", bufs=2, space="PSUM") as pap,
            tc.tile_pool(name="dram", bufs=1, space="DRAM") as dp,
        ):
            def load(d, shape, dtype, pool=wp, tag=None):
                t_ = pool.tile(shape, dtype, tag=tag or f"ld_{d.name}")
                nc.sync.dma_start(t_[:], d[:])
                return t_

            # persistent loads
            idx1_t = load(idx1_d, [128, T * 8], dt.int16)
            idx2_t = load(idx2_d, [128, T * 8], dt.int16)
            S_t = load(S_d, [128, T, 128], dt.bfloat16)
            wsrcT = load(wsrcT_d, [128, 4, 512], dt.bfloat16)
            wdstT = load(wdstT_d, [128, 2, 512], dt.bfloat16)
            w1lT = load(w1lT_d, [128, 4, 512], dt.bfloat16)
            w1rT = load(w1rT_d, [128, 4, 512], dt.bfloat16)
            w2lT = load(w2lT_d, [128, 4, 256], dt.bfloat16)
            w2rT = load(w2rT_d, [128, 4, 256], dt.bfloat16)
            xsT = load(xsT_d, [128, 4, LOC], dt.bfloat16, tag="xsT_rows")
            xdT = load(xdT_d, [128, 2, LOC], dt.bfloat16)
            recip_t = load(recip_d, [128, NB, 1], dt.float32)
            mask_t = load(mask_d, [1, LOC], dt.bfloat16)
            bsrcl_t = load(bsrcl_d, [1, 512], dt.bfloat16)
            bsrc_t = load(bsrc_d, [128, 4, 1], dt.float32)
            bdst_t = load(bdst_d, [128, 4, 1], dt.float32)
            gamma_t = load(gamma_d, [128, 4, 1], dt.float32)
            beta_t = load(beta_d, [128, 4, 1], dt.float32)
            b2_t = load(b2_d, [128, 2, 1], dt.float32)

            # big stream buffers
            hT = sp.tile([128, 4, COLS], dt.bfloat16, tag="actT")      # h feature-major
            r1T = sp.tile([128, 4, LOC], dt.float32, tag="bigf32a")    # x1 src half (pre-BN)
            m1rT = sp.tile([128, 4, LOC], dt.bfloat16, tag="mT")       # raw mean1^T
            m1T = sp.tile([128, 4, LOC], dt.bfloat16, tag="m1T")       # mean1 @ WsrcT
            x1dT = sp.tile([128, 4, LOC], dt.float32, tag="x1dT")      # x1 dst half (pre-BN)

            # ---------- h^T = [W_src x_src^T | W_dst x_dst^T] + biases ----------
            for t in range(4):
                for cs, cw in CH:
                    ps = pp.tile([128, 512], dt.float32, tag="pgemm")
                    for k in range(4):
                        nc.tensor.matmul(ps[:, :cw], wsrcT[:, k, t * 128:(t + 1) * 128],
                                         xsT[:, k, cs:cs + cw], start=(k == 0), stop=(k == 3))
                    nc.scalar.activation(hT[:, t, cs:cs + cw], ps[:, :cw], AF.Identity,
                                         bias=bsrc_t[:, t, :], scale=1.0)
            for t in range(4):
                for cs, cw in CH:
                    ps = pp.tile([128, 512], dt.float32, tag="pgemm")
                    for k in range(2):
                        nc.tensor.matmul(ps[:, :cw], wdstT[:, k, t * 128:(t + 1) * 128],
                                         xdT[:, k, cs:cs + cw], start=(k == 0), stop=(k == 1))
                    nc.scalar.activation(hT[:, t, LOC + cs:LOC + cs + cw], ps[:, :cw],
                                         AF.Identity, bias=bdst_t[:, t, :], scale=1.0)

            # ---------- r1^T src half = W1r h^T (src cols), f32 ----------
            for t in range(4):
                for cs, cw in CH:
                    ps = pp.tile([128, 512], dt.float32, tag="pgemm")
                    for k in range(4):
                        nc.tensor.matmul(ps[:, :cw], w1rT[:, k, t * 128:(t + 1) * 128],
                                         hT[:, k, cs:cs + cw], start=(k == 0), stop=(k == 3))
                    nc.vector.tensor_copy(r1T[:, t, cs:cs + cw], ps[:, :cw])

            # ---------- layer-1 aggregation: gather + S-matmul per block ----------
            def aggregate(idx_t, src_dram, outT, layer):
                # outT [128, 4, LOC] bf16 feature-major result (transposed mean)
                for b in range(NB):
                    pa = pap.tile([128, 512], dt.float32, tag="pagg")
                    for h in range(2):
                        ms = mp.tile([128, TB // 2, 512], dt.bfloat16, tag="msgs")
                        nc.gpsimd.dma_gather(
                            ms[:], src_dram[:],
                            idx_t[:, b * TB * 8 + h * TB * 4:b * TB * 8 + (h + 1) * TB * 4],
                            GH, GH, 512, queue_num=(2 * b + h) % 4)
                        for j in range(TB // 2):
                            jj = h * (TB // 2) + j
                            nc.tensor.matmul(pa[:], S_t[:, b * TB + jj, :], ms[:, j, :],
                                             start=(jj == 0), stop=(jj == TB - 1))
                    mb = meanp.tile([128, 512], dt.bfloat16, tag="meanblk")
                    nc.vector.tensor_scalar_mul(mb[:], pa[:], recip_t[:, b, :])
                    for t in range(4):
                        nc.sync.dma_start_transpose(
                            outT[:, t, b * 128:(b + 1) * 128], mb[:, t * 128:(t + 1) * 128])

            aggregate(idx1_t, x_src_bf, m1rT, 1)

            # ---------- mean1^T = W_src m1r^T + b_src (x) mask ----------
            for t in range(4):
                for cs, cw in CH:
                    ps = pp.tile([128, 512], dt.float32, tag="pgemm")
                    for k in range(4):
                        nc.tensor.matmul(ps[:, :cw], wsrcT[:, k, t * 128:(t + 1) * 128],
                                         m1rT[:, k, cs:cs + cw], start=(k == 0), stop=False)
                    nc.tensor.matmul(ps[:, :cw], bsrcl_t[0:1, t * 128:(t + 1) * 128],
                                     mask_t[0:1, cs:cs + cw], start=False, stop=True)
                    nc.vector.tensor_copy(m1T[:, t, cs:cs + cw], ps[:, :cw])

            # ---------- x1 dst half = W1r h^T(dst) + W1l mean1^T ----------
            for t in range(4):
                for cs, cw in CH:
                    ps = pp.tile([128, 512], dt.float32, tag="pgemm")
                    for k in range(4):
                        nc.tensor.matmul(ps[:, :cw], w1rT[:, k, t * 128:(t + 1) * 128],
                                         hT[:, k, LOC + cs:LOC + cs + cw], start=(k == 0), stop=False)
                    for k in range(4):
                        nc.tensor.matmul(ps[:, :cw], w1lT[:, k, t * 128:(t + 1) * 128],
                                         m1T[:, k, cs:cs + cw], start=False, stop=(k == 3))
                    nc.vector.tensor_copy(x1dT[:, t, cs:cs + cw], ps[:, :cw])

            # ---------- BN stats: S1, S2 over real columns ----------
            stats = sp.tile([128, 4, 4], dt.float32, tag="stats")   # s1s, s1d, s2s, s2d
            arin_sb = sp.tile([128, 4, 2], dt.float32, tag="arin")
            sq = sp.tile([128, LOC], dt.bfloat16, tag="sqscratch")
            for t in range(4):
                dst_real = x1dT[:, t, :].rearrange("p (b s) -> p b s", b=NB)[:, :, 0:CAP]
                sq_dst = sq[:, :].rearrange("p (b s) -> p b s", b=NB)[:, :, 0:CAP]
                nc.vector.tensor_reduce(stats[:, t, 0:1], r1T[:, t, 0:SRC_LOC],
                                        mybir.AxisListType.X, ALU.add)
                nc.vector.tensor_reduce(stats[:, t, 1:2], dst_real,
                                        mybir.AxisListType.XY, ALU.add)
                nc.scalar.activation(sq[:, 0:SRC_LOC], r1T[:, t, 0:SRC_LOC], AF.Square,
                                     accum_out=stats[:, t, 2:3])
                nc.scalar.activation(sq_dst, dst_real, AF.Square,
                                     accum_out=stats[:, t, 3:4])
                nc.vector.tensor_tensor(arin_sb[:, t, 0:1], stats[:, t, 0:1],
                                        stats[:, t, 1:2], ALU.add)
                nc.vector.tensor_tensor(arin_sb[:, t, 1:2], stats[:, t, 2:3],
                                        stats[:, t, 3:4], ALU.add)

            ar_in = dp.tile([128, 8], dt.float32)
            ar_out = dp.tile([128, 8], dt.float32, addr_space="Shared")
            nc.sync.dma_start(ar_in[:], arin_sb[:].rearrange("p a b -> p (a b)"))
            nc.gpsimd.collective_compute("AllReduce", ALU.add, replica_groups=RG,
                                         ins=[ar_in[:]], outs=[ar_out[:]])
            arsum = sp.tile([128, 4, 2], dt.float32, tag="arsum")
            nc.sync.dma_start(arsum[:], ar_out[:].rearrange("p (a b) -> p a b", a=4))

            # mean/var -> scale a, bias bb  (all [128, 4, 1] feature-major)
            mean_v = sp.tile([128, 4, 1], dt.float32, tag="vec1")
            var_v = sp.tile([128, 4, 1], dt.float32, tag="vec2")
            av = sp.tile([128, 4, 1], dt.float32, tag="vec3")
            bv = sp.tile([128, 4, 1], dt.float32, tag="vec4")
            inv_n = 1.0 / (N_SRC + N_DST)
            nc.vector.tensor_scalar_mul(mean_v[:], arsum[:, :, 0:1], inv_n)
            nc.vector.tensor_scalar_mul(var_v[:], arsum[:, :, 1:2], inv_n)   # E[x^2]
            nc.vector.tensor_tensor(av[:], mean_v[:], mean_v[:], ALU.mult)   # mean^2
            nc.vector.tensor_tensor(var_v[:], var_v[:], av[:], ALU.subtract)
            nc.vector.tensor_scalar_add(var_v[:], var_v[:], EPS)
            for t in range(4):
                nc.scalar.activation(var_v[:, t, :], var_v[:, t, :], AF.Sqrt, bias=0.0)
            nc.vector.reciprocal(var_v[:], var_v[:])                          # 1/std
            nc.vector.tensor_tensor(av[:], gamma_t[:], var_v[:], ALU.mult)    # a
            nc.vector.tensor_tensor(bv[:], mean_v[:], av[:], ALU.mult)
            nc.vector.tensor_tensor(bv[:], beta_t[:], bv[:], ALU.subtract)    # beta - mean*a

            # ---------- x1' = relu(a*x1 + b), bf16 (reuse hT slot via tag) ----------
            x1pT = sp.tile([128, 4, COLS], dt.bfloat16, tag="actT")
            for t in range(4):
                nc.scalar.activation(x1pT[:, t, 0:LOC], r1T[:, t, :], AF.Relu,
                                     bias=bv[:, t, :], scale=av[:, t, :])
                nc.scalar.activation(x1pT[:, t, LOC:COLS], x1dT[:, t, :], AF.Relu,
                                     bias=bv[:, t, :], scale=av[:, t, :])

            # ---------- transpose src half, AllGather ----------
            x1rows = sp.tile([128, NB, 512], dt.bfloat16, tag="xsT_rows")
            for t in range(4):
                for r in range(NB):
                    nc.sync.dma_start_transpose(
                        x1rows[:, r, t * 128:(t + 1) * 128], x1pT[:, t, r * 128:(r + 1) * 128])
            ag_in = dp.tile([LOC, 512], dt.bfloat16)
            ag_out = dp.tile([NC_ * LOC, 512], dt.bfloat16, addr_space="Shared")
            nc.sync.dma_start(ag_in[:].rearrange("(r p) f -> p r f", p=128), x1rows[:])
            nc.gpsimd.collective_compute("AllGather", ALU.bypass, replica_groups=RG,
                                         ins=[ag_in[:]], outs=[ag_out[:]])

            # ---------- layer-2 aggregation ----------
            m2T = sp.tile([128, 4, LOC], dt.bfloat16, tag="mT")
            aggregate(idx2_t, ag_out, m2T, 2)

            # ---------- output GEMMs ----------
            outT = sp.tile([128, 2, COLS], dt.float32, tag="bigf32a")
            for o in range(2):
                for cs, cw in CH:
                    ps = pp.tile([128, 512], dt.float32, tag="pgemm")
                    for k in range(4):
                        nc.tensor.matmul(ps[:, :cw], w2rT[:, k, o * 128:(o + 1) * 128],
                                         x1pT[:, k, cs:cs + cw], start=(k == 0), stop=(k == 3))
                    nc.scalar.activation(outT[:, o, cs:cs + cw], ps[:, :cw], AF.Identity,
                                         bias=b2_t[:, o, :], scale=1.0)
            for o in range(2):
                for cs, cw in CH:
                    ps = pp.tile([128, 512], dt.float32, tag="pgemm")
                    for k in range(4):
                        nc.tensor.matmul(ps[:, :cw], w2rT[:, k, o * 128:(o + 1) * 128],
                                         x1pT[:, k, LOC + cs:LOC + cs + cw],
                                         start=(k == 0), stop=False)
                    for k in range(4):
                        nc.tensor.matmul(ps[:, :cw], w2lT[:, k, o * 128:(o + 1) * 128],
                                         m2T[:, k, cs:cs + cw], start=False, stop=(k == 3))
                    nc.scalar.activation(outT[:, o, LOC + cs:LOC + cs + cw], ps[:, :cw],
                                         AF.Identity, bias=b2_t[:, o, :], scale=1.0)
            nc.sync.dma_start(out_d[:], outT[:])

    nc.compile()
    return nc


def kernel(**inputs):
    from concourse.bass_utils import run_bass_kernel_spmd

    x_src = np.asarray(inputs["x_src"], np.float32)
    x_dst = np.asarray(inputs["x_dst"], np.float32)
    edge_index = np.asarray(inputs["edge_index"])
    pre = _preprocess(edge_index)
    TB = pre["TB"]

    key = TB
    if key not in _BUILD_CACHE:
        _BUILD_CACHE[key] = _build(TB)
    nc = _BUILD_CACHE[key]

    x_src_bf = np.ascontiguousarray(x_src).astype(ml_dtypes.bfloat16)
    wsrcT = _w_tiles(inputs["W_src"])
    wdstT = _w_tiles(inputs["W_dst"])
    w1lT = _w_tiles(inputs["W1l"])
    w1rT = _w_tiles(inputs["W1r"])
    w2lT = _w_tiles(inputs["W2l"])
    w2rT = _w_tiles(inputs["W2r"])
    bsrc = _feat_major(inputs["b_src"], 4)
    bdst = _feat_major(inputs["b_dst"], 4)
    gamma = _feat_major(inputs["gamma"], 4)
    beta = _feat_major(inputs["beta"], 4)
    b2 = _feat_major(inputs["b2"], 2)
    bsrcl = np.asarray(inputs["b_src"], np.float32).reshape(1, 512).astype(ml_dtypes.bfloat16)

    in_maps = []
    for c in range(NC_):
        xs = x_src[c * SRC_LOC:(c + 1) * SRC_LOC]
        nodes = [nd for b in range(NB) for nd in
                 (pre["bin_nodes"][c * NB + b] + [None] * (128 - len(pre["bin_nodes"][c * NB + b])))]
        xd = np.zeros((LOC, IN_DST), np.float32)
        for col, nd in enumerate(nodes):
            if nd is not None:
                xd[col] = x_dst[nd]
        in_maps.append({
            "x_src_bf": x_src_bf,
            "xsT": _x_tiles(xs, LOC),
            "xdT": np.ascontiguousarray(
                xd.T.reshape(2, 128, LOC).transpose(1, 0, 2)).astype(ml_dtypes.bfloat16),
            "wsrcT": wsrcT, "wdstT": wdstT, "w1lT": w1lT, "w1rT": w1rT,
            "w2lT": w2lT, "w2rT": w2rT,
            "S": np.ascontiguousarray(pre["S"][c]),
            "idx1": pre["idx1"][c], "idx2": pre["idx2"][c],
            "recip": pre["recip"][c], "mask": pre["mask"][c],
            "bsrcl": bsrcl, "bsrc": bsrc, "bdst": bdst,
            "gamma": gamma, "beta": beta, "b2": b2,
        })

    res = run_bass_kernel_spmd(nc, in_maps, core_ids=list(range(NC_)))

    out = np.zeros((N_SRC + N_DST, OUT), np.float32)
    for c in range(NC_):
        arr = res.results[c]["outT"].transpose(1, 0, 2).reshape(OUT, COLS)  # [feat, col]
        out[c * SRC_LOC:(c + 1) * SRC_LOC] = arr[:, 0:SRC_LOC].T
        for b in range(NB):
            nodes = pre["bin_nodes"][c * NB + b]
            cols = LOC + b * 128 + np.arange(len(nodes))
            out[N_SRC + np.asarray(nodes, np.int64)] = arr[:, cols].T
    return out


# revision 7
# speedup vs baseline: 1.3335x; 1.2644x over previous
"""BipartiteSAGE (2-layer GraphSAGE on a bipartite graph) for 8 trn2 NeuronCores.

Strategy (dst-sharded, feature-major GEMMs):
- src rows sharded contiguously 1250/core; dst nodes assigned to 80 balanced
  (core, block) bins of 125 nodes via greedy binning so every 128-slot block
  has ~equal edge count.
- Layer-1 aggregation uses linearity: mean1_raw = segmean(x_src) gathered
  straight from a replicated bf16 copy of x_src; its transform is folded into
  Wfold = W1l @ W_src on the host (weights only). No collective for layer 1.
- Layer-2 gathers from an AllGather of the post-BN/ReLU src rows.
- Aggregation on device: dma_gather (128 edges/partition-tile) followed by
  one-hot S-matmul accumulation into PSUM per 128-dst block.
- Transposed means use single xbar-transpose calls per block; the 3D output
  lands in standard k-tile order (k = t*128+p).
- BatchNorm: local sum/sumsq reductions, AllReduce of [512,2] stats.
- All GEMMs bf16 (stationary = transposed weights), accumulation fp32 in PSUM.
"""

import numpy as np
import ml_dtypes

N_SRC, N_DST = 10000, 10000
IN_SRC, IN_DST, HID, OUT = 512, 256, 512, 256
N_EDGES = 160000
EPS = 1e-5
NC_ = 8            # cores
NB = 10            # dst blocks per core
CAP = 125          # dst nodes per bin
LOC = 1280         # padded local columns per half (src / dst)
COLS = 2 * LOC
SRC_LOC = N_SRC // NC_   # 1250


def _preprocess(edge_index):
    """Balanced dst binning + per-core edge tiles. Returns static structures."""
    src = np.asarray(edge_index[0], dtype=np.int64)
    dst = np.asarray(edge_index[1], dtype=np.int64) - N_SRC
    cnt = np.bincount(dst, minlength=N_DST)

    import heapq
    nbins = NC_ * NB
    order = np.argsort(-cnt, kind="stable")
    heap = [(0, b) for b in range(nbins)]
    heapq.heapify(heap)
    bin_nodes = [[] for _ in range(nbins)]
    bin_load = [0] * nbins
    for node in order:
        while True:
            load, b = heapq.heappop(heap)
            if len(bin_nodes[b]) < CAP:
                break
        bin_nodes[b].append(int(node))
        bin_load[b] = load + int(cnt[node])
        heapq.heappush(heap, (bin_load[b], b))

    bin_of = np.empty(N_DST, np.int64)
    slot_of = np.empty(N_DST, np.int64)
    for b, nodes in enumerate(bin_nodes):
        for s, nd in enumerate(nodes):
            bin_of[nd] = b
            slot_of[nd] = s

    ebin = bin_of[dst]
    order_e = np.lexsort((src, ebin))
    src_s, dst_s, ebin_s = src[order_e], dst[order_e], ebin[order_e]
    bounds = np.searchsorted(ebin_s, np.arange(nbins + 1))
    max_edges = max(bounds[b + 1] - bounds[b] for b in range(nbins))
    TB = int(np.ceil(max_edges / 128))           # tiles per block
    TB += TB % 2                                 # even (half-block gathers)
    T = NB * TB                                  # tiles per core

    idx1 = np.zeros((NC_, T * 128), np.int16)
    idx2 = np.zeros((NC_, T * 128), np.int16)
    S = np.zeros((NC_, 128, T, 128), ml_dtypes.bfloat16)
    recip = np.ones((NC_, 128, NB, 1), np.float32)
    mask = np.zeros((NC_, 1, LOC), ml_dtypes.bfloat16)

    for b in range(nbins):
        c, blk = divmod(b, NB)
        e0, e1 = bounds[b], bounds[b + 1]
        ss, dd = src_s[e0:e1], dst_s[e0:e1]
        n = e1 - e0
        base = blk * TB * 128
        idx1[c, base:base + n] = ss.astype(np.int16)
        idx2[c, base:base + n] = (1280 * (ss // SRC_LOC) + ss % SRC_LOC).astype(np.int16)
        pos = np.arange(n)
        S[c, pos % 128, blk * TB + pos // 128, slot_of[dd]] = 1.0
        for s, nd in enumerate(bin_nodes[b]):
            recip[c, s, blk, 0] = 1.0 / max(int(cnt[nd]), 1)
            if cnt[nd] > 0:
                mask[c, 0, blk * 128 + s] = 1.0

    def wrap(ix):  # flat [n] -> [128, n//16] wrapped-16 + replicated
        n = ix.shape[0]
        w = ix.reshape(n // 16, 16).T          # [16, n//16]
        return np.tile(w, (8, 1)).copy()

    idx1_w = np.stack([wrap(idx1[c]) for c in range(NC_)])
    idx2_w = np.stack([wrap(idx2[c]) for c in range(NC_)])
    return dict(TB=TB, T=T, bin_nodes=bin_nodes, cnt=cnt,
                idx1=idx1_w, idx2=idx2_w, S=S, recip=recip, mask=mask)


def _feat_major(v, kt):
    """[F] -> [128, kt, 1] f32 feature-major (f = t*128+p)."""
    return np.ascontiguousarray(
        np.asarray(v, np.float32).reshape(kt, 128, 1).transpose(1, 0, 2))


def _w_tiles(w):
    """W [out, in] -> lhsT tiles [128, in//128, out] bf16 (k = t*128+p)."""
    wt = np.asarray(w, np.float32).T           # [in, out]
    kin, kout = wt.shape
    return np.ascontiguousarray(
        wt.reshape(kin // 128, 128, kout).transpose(1, 0, 2)).astype(ml_dtypes.bfloat16)


def _x_tiles(x, ncols):
    """x [rows, F] -> rhs tiles [128, F//128, ncols] bf16 (feature-major, padded)."""
    r, f = x.shape
    xt = np.zeros((f, ncols), np.float32)
    xt[:, :r] = np.asarray(x, np.float32).T
    return np.ascontiguousarray(
        xt.reshape(f // 128, 128, ncols).transpose(1, 0, 2)).astype(ml_dtypes.bfloat16)


_BUILD_CACHE = {}


def _build(TB):
    import concourse.bacc as bacc
    import concourse.mybir as mybir
    from concourse import tile

    dt = mybir.dt
    T = NB * TB
    GH = TB * 64                  # idxs per half-block gather
    CH = [(0, 512), (512, 512), (1024, 256)]   # chunks over a 1280 half

    nc = bacc.Bacc("TRN2", target_bir_lowering=False, debug=False, num_devices=NC_,
                   num_swdge_queues=4)

    # ---- external inputs ----
    x_src_bf = nc.dram_tensor("x_src_bf", [N_SRC, 512], dt.bfloat16, kind="ExternalInput")
    xsT_d = nc.dram_tensor("xsT", [128, 4, LOC], dt.bfloat16, kind="ExternalInput")
    xdT_d = nc.dram_tensor("xdT", [128, 2, LOC], dt.bfloat16, kind="ExternalInput")
    wsrcT_d = nc.dram_tensor("wsrcT", [128, 4, 512], dt.bfloat16, kind="ExternalInput")
    wdstT_d = nc.dram_tensor("wdstT", [128, 2, 512], dt.bfloat16, kind="ExternalInput")
    wfoldT_d = nc.dram_tensor("wfoldT", [128, 4, 512], dt.bfloat16, kind="ExternalInput")
    w1rT_d = nc.dram_tensor("w1rT", [128, 4, 512], dt.bfloat16, kind="ExternalInput")
    w2lT_d = nc.dram_tensor("w2lT", [128, 4, 256], dt.bfloat16, kind="ExternalInput")
    w2rT_d = nc.dram_tensor("w2rT", [128, 4, 256], dt.bfloat16, kind="ExternalInput")
    S_d = nc.dram_tensor("S", [128, T, 128], dt.bfloat16, kind="ExternalInput")
    idx1_d = nc.dram_tensor("idx1", [128, T * 8], dt.int16, kind="ExternalInput")
    idx2_d = nc.dram_tensor("idx2", [128, T * 8], dt.int16, kind="ExternalInput")
    recip_d = nc.dram_tensor("recip", [128, NB, 1], dt.float32, kind="ExternalInput")
    mask_d = nc.dram_tensor("mask", [1, LOC], dt.bfloat16, kind="ExternalInput")
    bsrcl_d = nc.dram_tensor("bsrcl", [1, 512], dt.bfloat16, kind="ExternalInput")
    bsrc_d = nc.dram_tensor("bsrc", [128, 4, 1], dt.float32, kind="ExternalInput")
    bdst_d = nc.dram_tensor("bdst", [128, 4, 1], dt.float32, kind="ExternalInput")
    gamma_d = nc.dram_tensor("gamma", [128, 4, 1], dt.float32, kind="ExternalInput")
    beta_d = nc.dram_tensor("beta", [128, 4, 1], dt.float32, kind="ExternalInput")
    b2_d = nc.dram_tensor("b2", [128, 2, 1], dt.float32, kind="ExternalInput")
    out_d = nc.dram_tensor("outT", [128, 2, COLS], dt.float32, kind="ExternalOutput")

    RG = [list(range(NC_))]
    AF = mybir.ActivationFunctionType
    ALU = mybir.AluOpType

    with tile.TileContext(nc) as tc:
        with (
            tc.tile_pool(name="w", bufs=1) as wp,
            tc.tile_pool(name="st", bufs=1) as sp,
            tc.tile_pool(name="msgs", bufs=4) as mp,
            tc.tile_pool(name="mean", bufs=3) as meanp,
            tc.tile_pool(name="ps", bufs=5, space="PSUM") as pp,
            tc.tile_pool(name="pagg", bufs=2, space="PSUM") as pap,
            tc.tile_pool(name="dram", bufs=1, space="DRAM") as dp,
        ):
            def load(d, shape, dtype, pool=wp, tag=None):
                t_ = pool.tile(shape, dtype, tag=tag or f"ld_{d.name}")
                nc.sync.dma_start(t_[:], d[:])
                return t_

            # persistent loads (gather-critical first)
            idx1_t = load(idx1_d, [128, T * 8], dt.int16)
            S_t = load(S_d, [128, T, 128], dt.bfloat16)
            recip_t = load(recip_d, [128, NB, 1], dt.float32)
            idx2_t = load(idx2_d, [128, T * 8], dt.int16)
            wsrcT = load(wsrcT_d, [128, 4, 512], dt.bfloat16)
            wdstT = load(wdstT_d, [128, 2, 512], dt.bfloat16)
            wfoldT = load(wfoldT_d, [128, 4, 512], dt.bfloat16)
            w1rT = load(w1rT_d, [128, 4, 512], dt.bfloat16)
            w2lT = load(w2lT_d, [128, 4, 256], dt.bfloat16)
            w2rT = load(w2rT_d, [128, 4, 256], dt.bfloat16)
            xsT = load(xsT_d, [128, 4, LOC], dt.bfloat16, tag="xsT_rows")
            xdT = load(xdT_d, [128, 2, LOC], dt.bfloat16)
            mask_t = load(mask_d, [1, LOC], dt.bfloat16)
            bsrcl_t = load(bsrcl_d, [1, 512], dt.bfloat16)
            bsrc_t = load(bsrc_d, [128, 4, 1], dt.float32)
            bdst_t = load(bdst_d, [128, 4, 1], dt.float32)
            gamma_t = load(gamma_d, [128, 4, 1], dt.float32)
            beta_t = load(beta_d, [128, 4, 1], dt.float32)
            b2_t = load(b2_d, [128, 2, 1], dt.float32)

            hT = sp.tile([128, 4, COLS], dt.bfloat16, tag="actT")      # h feature-major
            r1T = sp.tile([128, 4, LOC], dt.float32, tag="bigf32a")    # x1 src half (pre-BN)
            m1rT = sp.tile([128, NB, 4, 128], dt.bfloat16, tag="mT")   # mean1^T (k=4p+t)
            x1dT = sp.tile([128, 4, LOC], dt.float32, tag="x1dT")      # x1 dst half (pre-BN)

            # ---------- aggregation: gather + S-matmul per block ----------
            def aggregate(idx_t, src_dram, outT):
                # outT [128, NB, 4, 128] bf16: transposed mean, k-interleaved
                for b in range(NB):
                    pa = pap.tile([128, 512], dt.float32, tag="pagg")
                    for h in range(2):
                        ms = mp.tile([128, TB // 2, 512], dt.bfloat16, tag="msgs")
                        nc.gpsimd.dma_gather(
                            ms[:], src_dram[:],
                            idx_t[:, b * TB * 8 + h * TB * 4:b * TB * 8 + (h + 1) * TB * 4],
                            GH, GH, 512, queue_num=(2 * b + h) % 4)
                        for j in range(TB // 2):
                            jj = h * (TB // 2) + j
                            nc.tensor.matmul(pa[:], S_t[:, b * TB + jj, :], ms[:, j, :],
                                             start=(jj == 0), stop=(jj == TB - 1))
                    mb = meanp.tile([128, 512], dt.bfloat16, tag="meanblk")
                    nc.vector.tensor_scalar_mul(mb[:], pa[:], recip_t[:, b, :])
                    nc.sync.dma_start_transpose(outT[:, b, :, :], mb[:])

            aggregate(idx1_t, x_src_bf, m1rT)

            # ---------- h^T = [W_src x_src^T | W_dst x_dst^T] + biases ----------
            for t in range(4):
                for cs, cw in CH:
                    ps = pp.tile([128, 512], dt.float32, tag="pgemm")
                    for k in range(4):
                        nc.tensor.matmul(ps[:, :cw], wsrcT[:, k, t * 128:(t + 1) * 128],
                                         xsT[:, k, cs:cs + cw], start=(k == 0), stop=(k == 3))
                    nc.scalar.activation(hT[:, t, cs:cs + cw], ps[:, :cw], AF.Identity,
                                         bias=bsrc_t[:, t, :], scale=1.0)
            for t in range(4):
                for cs, cw in CH:
                    ps = pp.tile([128, 512], dt.float32, tag="pgemm")
                    for k in range(2):
                        nc.tensor.matmul(ps[:, :cw], wdstT[:, k, t * 128:(t + 1) * 128],
                                         xdT[:, k, cs:cs + cw], start=(k == 0), stop=(k == 1))
                    nc.scalar.activation(hT[:, t, LOC + cs:LOC + cs + cw], ps[:, :cw],
                                         AF.Identity, bias=bdst_t[:, t, :], scale=1.0)

            # ---------- r1^T src half = W1r h^T (src cols), f32 ----------
            for t in range(4):
                for cs, cw in CH:
                    ps = pp.tile([128, 512], dt.float32, tag="pgemm")
                    for k in range(4):
                        nc.tensor.matmul(ps[:, :cw], w1rT[:, k, t * 128:(t + 1) * 128],
                                         hT[:, k, cs:cs + cw], start=(k == 0), stop=(k == 3))
                    nc.vector.tensor_copy(r1T[:, t, cs:cs + cw], ps[:, :cw])

            # ---------- x1 dst half = W1r h^T(dst) + Wfold mean1^T + bsrc1l(x)mask ----
            for t in range(4):
                for ci, (cs, cw) in enumerate(CH):
                    b0, nb_c = 4 * ci, cw // 128
                    ps = pp.tile([128, 512], dt.float32, tag="pgemm")
                    for k in range(4):
                        nc.tensor.matmul(ps[:, :cw], w1rT[:, k, t * 128:(t + 1) * 128],
                                         hT[:, k, LOC + cs:LOC + cs + cw], start=(k == 0), stop=False)
                    for k in range(4):
                        nc.tensor.matmul(ps[:, :cw], wfoldT[:, k, t * 128:(t + 1) * 128],
                                         m1rT[:, b0:b0 + nb_c, k, :], start=False, stop=False)
                    nc.tensor.matmul(ps[:, :cw], bsrcl_t[0:1, t * 128:(t + 1) * 128],
                                     mask_t[0:1, cs:cs + cw], start=False, stop=True)
                    nc.vector.tensor_copy(x1dT[:, t, cs:cs + cw], ps[:, :cw])

            # ---------- BN stats: S1, S2 over real columns ----------
            stats = sp.tile([128, 4, 4], dt.float32, tag="stats")
            arin_sb = sp.tile([128, 4, 2], dt.float32, tag="arin")
            sq = sp.tile([128, LOC], dt.bfloat16, tag="sqscratch")
            for t in range(4):
                dst_real = x1dT[:, t, :].rearrange("p (b s) -> p b s", b=NB)[:, :, 0:CAP]
                sq_dst = sq[:, :].rearrange("p (b s) -> p b s", b=NB)[:, :, 0:CAP]
                nc.vector.tensor_reduce(stats[:, t, 0:1], r1T[:, t, 0:SRC_LOC],
                                        mybir.AxisListType.X, ALU.add)
                nc.vector.tensor_reduce(stats[:, t, 1:2], dst_real,
                                        mybir.AxisListType.XY, ALU.add)
                nc.scalar.activation(sq[:, 0:SRC_LOC], r1T[:, t, 0:SRC_LOC], AF.Square,
                                     accum_out=stats[:, t, 2:3])
                nc.scalar.activation(sq_dst, dst_real, AF.Square,
                                     accum_out=stats[:, t, 3:4])
                nc.vector.tensor_tensor(arin_sb[:, t, 0:1], stats[:, t, 0:1],
                                        stats[:, t, 1:2], ALU.add)
                nc.vector.tensor_tensor(arin_sb[:, t, 1:2], stats[:, t, 2:3],
                                        stats[:, t, 3:4], ALU.add)

            ar_in = dp.tile([128, 8], dt.float32)
            ar_out = dp.tile([128, 8], dt.float32, addr_space="Shared")
            nc.sync.dma_start(ar_in[:], arin_sb[:].rearrange("p a b -> p (a b)"))
            nc.gpsimd.collective_compute("AllReduce", ALU.add, replica_groups=RG,
                                         ins=[ar_in[:]], outs=[ar_out[:]])
            arsum = sp.tile([128, 4, 2], dt.float32, tag="arsum")
            nc.sync.dma_start(arsum[:], ar_out[:].rearrange("p (a b) -> p a b", a=4))

            mean_v = sp.tile([128, 4, 1], dt.float32, tag="vec1")
            var_v = sp.tile([128, 4, 1], dt.float32, tag="vec2")
            av = sp.tile([128, 4, 1], dt.float32, tag="vec3")
            bv = sp.tile([128, 4, 1], dt.float32, tag="vec4")
            inv_n = 1.0 / (N_SRC + N_DST)
            nc.vector.tensor_scalar_mul(mean_v[:], arsum[:, :, 0:1], inv_n)
            nc.vector.tensor_scalar_mul(var_v[:], arsum[:, :, 1:2], inv_n)
            nc.vector.tensor_tensor(av[:], mean_v[:], mean_v[:], ALU.mult)
            nc.vector.tensor_tensor(var_v[:], var_v[:], av[:], ALU.subtract)
            nc.vector.tensor_scalar_add(var_v[:], var_v[:], EPS)
            for t in range(4):
                nc.scalar.activation(var_v[:, t, :], var_v[:, t, :], AF.Sqrt, bias=0.0)
            nc.vector.reciprocal(var_v[:], var_v[:])
            nc.vector.tensor_tensor(av[:], gamma_t[:], var_v[:], ALU.mult)
            nc.vector.tensor_tensor(bv[:], mean_v[:], av[:], ALU.mult)
            nc.vector.tensor_tensor(bv[:], beta_t[:], bv[:], ALU.subtract)

            # ---------- x1' = relu(a*x1 + b), bf16 ----------
            x1pT = sp.tile([128, 4, COLS], dt.bfloat16, tag="actT")
            for t in range(4):
                nc.scalar.activation(x1pT[:, t, 0:LOC], r1T[:, t, :], AF.Relu,
                                     bias=bv[:, t, :], scale=av[:, t, :])
                nc.scalar.activation(x1pT[:, t, LOC:COLS], x1dT[:, t, :], AF.Relu,
                                     bias=bv[:, t, :], scale=av[:, t, :])

            # ---------- output src half (fills the AllGather bubble) ----------
            outT = sp.tile([128, 2, COLS], dt.float32, tag="bigf32a")
            for o in range(2):
                for cs, cw in CH:
                    ps = pp.tile([128, 512], dt.float32, tag="pgemm")
                    for k in range(4):
                        nc.tensor.matmul(ps[:, :cw], w2rT[:, k, o * 128:(o + 1) * 128],
                                         x1pT[:, k, cs:cs + cw], start=(k == 0), stop=(k == 3))
                    nc.scalar.activation(outT[:, o, cs:cs + cw], ps[:, :cw], AF.Identity,
                                         bias=b2_t[:, o, :], scale=1.0)

            # ---------- transpose src half, AllGather ----------
            x1rows = sp.tile([128, 12, 512], dt.bfloat16, tag="xsT_rows")
            for t in range(4):
                for q, (qs, qw) in enumerate(CH):
                    ntt = qw // 128
                    nc.sync.dma_start_transpose(
                        x1rows[:, 4 * q:4 * q + ntt, t * 128:(t + 1) * 128],
                        x1pT[:, t, qs:qs + qw])
            ag_in = dp.tile([LOC, 512], dt.bfloat16)
            ag_out = dp.tile([NC_ * LOC, 512], dt.bfloat16, addr_space="Shared")
            for q, (qs, qw) in enumerate(CH):
                ntt = qw // 128
                nc.sync.dma_start(
                    ag_in[qs:qs + qw, :].rearrange("(t p) f -> p t f", p=128),
                    x1rows[:, 4 * q:4 * q + ntt, :])
            nc.gpsimd.collective_compute("AllGather", ALU.bypass, replica_groups=RG,
                                         ins=[ag_in[:]], outs=[ag_out[:]])

            # ---------- layer-2 aggregation ----------
            m2T = sp.tile([128, NB, 4, 128], dt.bfloat16, tag="mT")
            aggregate(idx2_t, ag_out, m2T)

            # ---------- output dst half ----------
            for o in range(2):
                for ci, (cs, cw) in enumerate(CH):
                    b0, nb_c = 4 * ci, cw // 128
                    ps = pp.tile([128, 512], dt.float32, tag="pgemm")
                    for k in range(4):
                        nc.tensor.matmul(ps[:, :cw], w2rT[:, k, o * 128:(o + 1) * 128],
                                         x1pT[:, k, LOC + cs:LOC + cs + cw],
                                         start=(k == 0), stop=False)
                    for k in range(4):
                        nc.tensor.matmul(ps[:, :cw], w2lT[:, k, o * 128:(o + 1) * 128],
                                         m2T[:, b0:b0 + nb_c, k, :], start=False, stop=(k == 3))
                    nc.scalar.activation(outT[:, o, LOC + cs:LOC + cs + cw], ps[:, :cw],
                                         AF.Identity, bias=b2_t[:, o, :], scale=1.0)
            nc.sync.dma_start(out_d[:], outT[:])

    nc.compile()
    return nc


def kernel(**inputs):
    from concourse.bass_utils import run_bass_kernel_spmd

    x_src = np.asarray(inputs["x_src"], np.float32)
    x_dst = np.asarray(inputs["x_dst"], np.float32)
    edge_index = np.asarray(inputs["edge_index"])
    pre = _preprocess(edge_index)
    TB = pre["TB"]

    key = TB
    if key not in _BUILD_CACHE:
        _BUILD_CACHE[key] = _build(TB)
    nc = _BUILD_CACHE[key]

    W_src = np.asarray(inputs["W_src"], np.float32)
    W1l = np.asarray(inputs["W1l"], np.float32)
    wfold = W1l @ W_src                       # [512, 512] host weight fold
    bsrc1l = W1l @ np.asarray(inputs["b_src"], np.float32)

    x_src_bf = np.ascontiguousarray(x_src).astype(ml_dtypes.bfloat16)
    wsrcT = _w_tiles(W_src)
    wdstT = _w_tiles(inputs["W_dst"])
    wfoldT = _w_tiles(wfold)
    w1rT = _w_tiles(inputs["W1r"])
    w2lT = _w_tiles(inputs["W2l"])
    w2rT = _w_tiles(inputs["W2r"])
    bsrc = _feat_major(inputs["b_src"], 4)
    bdst = _feat_major(inputs["b_dst"], 4)
    gamma = _feat_major(inputs["gamma"], 4)
    beta = _feat_major(inputs["beta"], 4)
    b2 = _feat_major(inputs["b2"], 2)
    bsrcl = bsrc1l.reshape(1, 512).astype(ml_dtypes.bfloat16)

    in_maps = []
    for c in range(NC_):
        xs = x_src[c * SRC_LOC:(c + 1) * SRC_LOC]
        nodes = [nd for b in range(NB) for nd in
                 (pre["bin_nodes"][c * NB + b] + [None] * (128 - len(pre["bin_nodes"][c * NB + b])))]
        xd = np.zeros((LOC, IN_DST), np.float32)
        for col, nd in enumerate(nodes):
            if nd is not None:
                xd[col] = x_dst[nd]
        in_maps.append({
            "x_src_bf": x_src_bf,
            "xsT": _x_tiles(xs, LOC),
            "xdT": np.ascontiguousarray(
                xd.T.reshape(2, 128, LOC).transpose(1, 0, 2)).astype(ml_dtypes.bfloat16),
            "wsrcT": wsrcT, "wdstT": wdstT, "wfoldT": wfoldT, "w1rT": w1rT,
            "w2lT": w2lT, "w2rT": w2rT,
            "S": np.ascontiguousarray(pre["S"][c]),
            "idx1": pre["idx1"][c], "idx2": pre["idx2"][c],
            "recip": pre["recip"][c], "mask": pre["mask"][c],
            "bsrcl": bsrcl, "bsrc": bsrc, "bdst": bdst,
            "gamma": gamma, "beta": beta, "b2": b2,
        })

    res = run_bass_kernel_spmd(nc, in_maps, core_ids=list(range(NC_)))

    out = np.zeros((N_SRC + N_DST, OUT), np.float32)
    for c in range(NC_):
        arr = res.results[c]["outT"].transpose(1, 0, 2).reshape(OUT, COLS)
        out[c * SRC_LOC:(c + 1) * SRC_LOC] = arr[:, 0:SRC_LOC].T
        for b in range(NB):
            nodes = pre["bin_nodes"][c * NB + b]
            cols = LOC + b * 128 + np.arange(len(nodes))
            out[N_SRC + np.asarray(nodes, np.int64)] = arr[:, cols].T
    return out
